# revision 2
# baseline (speedup 1.0000x reference)
"""Trainium2 Bass kernel for the AttnBlock-style attention module.

Reference computation (note softmax over axis=1, the *i* axis):
    q = wq @ x + bq ; k = wk @ x + bk ; v = wv @ x + bv      (per-pixel 1x1 conv)
    s[b,i,j] = (q[b,:,i] . k[b,:,j]) * C**-0.5
    attn = softmax_i(s)                                      (normalize over i!)
    out[b,c,i] = sum_j attn[b,i,j] v[b,c,j]
    y = wp @ out + bp

Sharding: 8 cores = 4 batches x 2 j-halves. The softmax over i is local to a
j-split (it normalizes each attention *column* j over all i). Each core gets x
with its j-half rotated to columns 0..2047 (a pure permutation of the pixel
axis, which passes through every per-pixel op and the i-softmax unchanged; the
host un-rotates the partial output). Each core:
  - computes q for all N=4096 pixels, k/v for columns 0..2047,
  - s_T[j, i] = k^T q   (j on partitions -> softmax reduction is free-axis),
  - attn = exp(s/16) stored unnormalized in bf16; per-j denominators D[j]
    from the fused activation accum_out; 1/D folded into v rows,
  - out_partial[c, i] = sum_{j in half} v_scaled[c,j] attn_T[j,i],
  - y_partial = wp @ out_partial   (bias bp added on host).
Host un-rotates and sums the two j-half partials per batch and adds bp.

DMA layout: x / weights / biases are pre-packed on the host into the exact
SBUF tile layouts so every DMA moves 2-4KB contiguous runs per partition
(128 descriptors per block instead of 256+ sub-1KB ones), and the block DMAs
alternate between the two hardware DGE queues (SP + Activation) so descriptor
processing overlaps. The final out-accumulation group is split (jts 12-14 /
jt 15) so only a 2-matmul chain per output slice remains after the last exp,
letting the y stores pipeline out instead of stacking at the end.
"""

import numpy as np

import concourse.bass as bass
import concourse.mybir as mybir
import concourse.tile as tile
from concourse import bacc
from concourse import bass_utils

P = 128
B = 4
C = 256
N = 4096          # 64*64 pixels
NJ = 2048         # j columns per core
NJT = NJ // P     # 16 j tiles
SCALE = 1.0 / np.sqrt(C).item()   # 1/16

F32 = mybir.dt.float32
BF16 = mybir.dt.bfloat16
AF = mybir.ActivationFunctionType

# x column blocks: (lo, width); packed host-side as [128, 2, w] per block
XBLK = [(0, 512), (512, 512), (1024, 1024), (2048, 1024), (3072, 1024)]
XOFF = [0]
for _lo, _w in XBLK:
    XOFF.append(XOFF[-1] + 2 * _w)      # free-dim offset into xp rows


def _build_module():
    nc = bacc.Bacc("TRN2", target_bir_lowering=False, debug=False, num_devices=8)

    xp_t = nc.dram_tensor("xp", [P, 2 * N], BF16, kind="ExternalInput")
    wp_t = nc.dram_tensor("wp", [P, 8 * C], BF16, kind="ExternalInput")
    bp_t = nc.dram_tensor("bp", [P, 4 + C], F32, kind="ExternalInput")
    y_t = nc.dram_tensor("y", [C, N], F32, kind="ExternalOutput")

    with tile.TileContext(nc) as tc:
        _emit(nc, tc, xp_t, wp_t, bp_t, y_t)
    nc.compile()
    return nc


def _emit(nc, tc, xp_t, wp_t, bp_t, y_t):
    from contextlib import ExitStack

    with ExitStack() as top:
        const = top.enter_context(tc.tile_pool(name="const", bufs=1))
        big = top.enter_context(tc.tile_pool(name="big", bufs=1))

        # ---- constants: host-packed, one DMA each -----------------------
        # w_all[:, 2*w + ci, :] = rows ci*128.. of weight w's transpose
        # slot order (host side): wq(0,1), wk(2,3), w2=wp@wv(4,5), pad(6,7)
        w_all = const.tile([P, 8, C], BF16, tag="w_all", name="w_all")
        nc.sync.dma_start(w_all[:], bp_ap(wp_t, [[8 * C, P], [1, 8 * C]]))

        b_pack = const.tile([P, 4 + C], F32, tag="b_pack", name="b_pack")
        nc.scalar.dma_start(b_pack[:], bp_ap(bp_t, [[4 + C, P], [1, 4 + C]]))
        b_all = b_pack[:, 0:4]       # cols: 0,1 = bq halves; 2,3 = bk halves
        bv_sb = b_pack[:, 4:4 + C]   # w2-folded v bias, broadcast to partitions

        def wslice(w, ci, ch):   # lhsT [128 ci, 128 co] for co half ch
            return w_all[:, 2 * w + ci, ch * P:(ch + 1) * P]

        # ---- persistent activations -----------------------------------
        q_bf = [big.tile([P, N], BF16, tag=f"q{ch}", name=f"q{ch}") for ch in range(2)]
        k_bf = [big.tile([P, NJ], BF16, tag=f"k{ch}", name=f"k{ch}") for ch in range(2)]
        v_all = big.tile([P, NJT, C], BF16, tag="v_all", name="v_all")
        attn = [big.tile([P, N], BF16, tag=f"a{jt}", name=f"a{jt}") for jt in range(NJT)]
        # cols 0:64 = per-(jt,iq) exp sums, 64:80 = D, 80:96 = 1/D
        d_all = big.tile([P, 96], F32, tag="d_all", name="d_all")
        dsum_all = d_all[:, 64:96]

        # ---- warmups: run while the x DMA streams in -------------------
        # ~8 dummy matmuls lift the PE HAM clock-gate toward 8/8 before real
        # work arrives, and a dummy Exp pulls the ~2.7us ACT table load off
        # the critical path of the first score tile.
        with tc.tile_pool(name="warm", bufs=1) as wp_pool, \
             tc.tile_pool(name="warm_ps", bufs=1, space="PSUM") as wpp:
            wsb = wp_pool.tile([P, 512], BF16, tag="wsb", name="wsb")
            wex = wsb[:, 508:509]
            wps = wpp.tile([P, 512], F32, tag="wps", name="wps")
            nc.vector.memset(wsb[:], 0.0)
            for _ in range(8):
                nc.tensor.matmul(wps[:], wsb[:, 0:P], wsb[:],
                                 start=True, stop=True)
            nc.scalar.activation(wex[:], wps[:, 0:1], AF.Exp, scale=0.0)

        def bias_store(out_ap, ps, bias_ap, on_act):
            if on_act:
                nc.scalar.activation(out_ap, ps, AF.Identity, bias=bias_ap)
            else:
                nc.vector.tensor_scalar_add(out_ap, ps, bias_ap)

        psp = top.enter_context(tc.tile_pool(name="ps_s", bufs=2, space="PSUM"))

        def s_tile(jt, iq):
            # one [128,1024] score tile + exp(+accum) into the attn store
            ps = psp.tile([P, 1024], F32, tag="s", name="s_ps")
            for ch in range(2):
                lhs = k_bf[ch][:, jt * P:(jt + 1) * P]
                for t in range(2):
                    nc.tensor.matmul(
                        ps[:, t * 512:(t + 1) * 512], lhs,
                        q_bf[ch][:, iq * 1024 + t * 512: iq * 1024 + (t + 1) * 512],
                        start=(ch == 0), stop=(ch == 1),
                    )
            nc.scalar.activation(
                attn[jt][:, iq * 1024:(iq + 1) * 1024], ps[:],
                AF.Exp, scale=float(SCALE),
                accum_out=d_all[:, jt * 4 + iq: jt * 4 + iq + 1],
            )

        with tc.tile_pool(name="xload", bufs=1) as xp:
            # x arrives in packed [128, 2, w] column blocks, alternating
            # between the SP and ACT hardware DGE queues.
            xb = [xp.tile([P, 2, w], BF16, tag=f"xb{b}", name=f"xb{b}")
                  for b, (lo, w) in enumerate(XBLK)]
            for b, (lo, w) in enumerate(XBLK):
                eng = nc.sync if b % 2 == 0 else nc.scalar
                eng.dma_start(
                    xb[b][:],
                    bp_ap(xp_t, [[2 * N, P], [1, 2 * w]], off=XOFF[b]),
                )

            def xsl(ci, lo, size):
                # x[ci*128:(ci+1)*128, lo:lo+size] as an AP (within one block)
                for b, (blo, w) in enumerate(XBLK):
                    if blo <= lo and lo + size <= blo + w:
                        return xb[b][:, ci, lo - blo:lo - blo + size]
                raise AssertionError((lo, size))

            # ---- phase 1: k, q, vp projections, emitted block-wise -------
            # Work is ordered by which x column-block it needs, so the PE
            # starts as soon as block 0 lands and never waits for later
            # blocks.
            with tc.tile_pool(name="ps_qkv", bufs=4, space="PSUM") as pq:
                for blk in range(4):
                    if blk < 2:
                        # k chunks of this block (k covers columns 0..NJ)
                        for ch in range(2):
                            pss = [pq.tile([P, 512], F32, tag="ps", name="ps") for _ in range(2)]
                            for ci in range(2):
                                lhs = wslice(1, ci, ch)
                                for t2 in range(2):
                                    t = blk * 2 + t2
                                    nc.tensor.matmul(
                                        pss[t2][:], lhs,
                                        xsl(ci, t * 512, 512),
                                        start=(ci == 0), stop=(ci == 1),
                                    )
                            for t2 in range(2):
                                t = blk * 2 + t2
                                bias_store(k_bf[ch][:, t * 512:(t + 1) * 512], pss[t2][:],
                                           b_all[:, 2 + ch:3 + ch], on_act=(ch == 0))
                    # q chunks of this block
                    for ch in range(2):
                        pss = [pq.tile([P, 512], F32, tag="ps", name="ps") for _ in range(2)]
                        for ci in range(2):
                            lhs = wslice(0, ci, ch)
                            for t2 in range(2):
                                ic = blk * 2 + t2
                                nc.tensor.matmul(
                                    pss[t2][:], lhs,
                                    xsl(ci, ic * 512, 512),
                                    start=(ci == 0), stop=(ci == 1),
                                )
                        for t2 in range(2):
                            ic = blk * 2 + t2
                            bias_store(q_bf[ch][:, ic * 512:(ic + 1) * 512], pss[t2][:],
                                       b_all[:, ch:ch + 1], on_act=(ch == 0))
                    if blk == 1:
                        # k and the first 4 q chunks exist: start the jt0-3
                        # score tiles now so ACT's exp backlog begins while
                        # the PE finishes the projections (ACT is idle here)
                        for jt0 in range(4):
                            for iq0 in range(2):
                                s_tile(jt0, iq0)
                    elif blk == 2:
                        for jt0 in range(4):
                            s_tile(jt0, 2)
                    if blk < 2:
                        # vp_T[j, co] for this block's 8 j-tiles (wp folded
                        # into v on the host: W2 = wp@wv, b2 = wp@bv)
                        for jtg in range(2):
                            pss = [pq.tile([P, C], F32, tag="ps", name="ps") for _ in range(4)]
                            for ci in range(2):
                                for t in range(4):
                                    jt = blk * 8 + jtg * 4 + t
                                    nc.tensor.matmul(
                                        pss[t][:],
                                        xsl(ci, jt * P, P),
                                        w_all[:, 2 * 2 + ci, :],
                                        start=(ci == 0), stop=(ci == 1),
                                    )
                            for t in range(4):
                                nc.vector.tensor_add(
                                    v_all[:, blk * 8 + jtg * 4 + t, :], pss[t][:], bv_sb[:]
                                )

        # ---- phase 2+3 fused: scores/exp interleaved with y accum ------
        # s tiles are [128, 1024] (2 PSUM banks); the attention-weighted y
        # accumulation runs in j-groups, SBUF-accumulated.  Groups are
        # jts 0-3 / 4-7 / 8-11 / 12-14 / 15: the last group is a single
        # 2-matmul chain per slice, so after the final exp only ~16 small
        # matmuls + adds gate the y stores (which alternate DGE queues).
        with tc.tile_pool(name="yaccp", bufs=1) as yp, \
             tc.tile_pool(name="ps_o", bufs=2, space="PSUM") as po, \
             tc.tile_pool(name="ysb", bufs=2) as ysb_pool:
            y_acc = yp.tile([P, 8, 1024], F32, tag="y_acc", name="y_acc")

            def out_chain(jts, gi, idx, final=False):
                # one accumulation chain: attn[jts] @ v[jts] into (iq, ch)
                iq, ch = divmod(idx, 2)
                ops = po.tile([P, 1024], F32, tag="og", name="og")
                for n, j2 in enumerate(jts):
                    lhs = v_all[:, j2, ch * P:(ch + 1) * P]
                    for t in range(2):
                        nc.tensor.matmul(
                            ops[:, t * 512:(t + 1) * 512], lhs,
                            attn[j2][:, iq * 1024 + t * 512: iq * 1024 + (t + 1) * 512],
                            start=(n == 0), stop=(n == len(jts) - 1),
                        )
                if gi == 0:
                    nc.vector.tensor_copy(y_acc[:, idx, :], ops[:])
                elif not final:
                    nc.vector.tensor_add(y_acc[:, idx, :], ops[:], y_acc[:, idx, :])
                else:
                    y_sb = ysb_pool.tile([P, 1024], F32, tag="ysb", name="ysb")
                    nc.vector.tensor_add(y_sb[:], ops[:], y_acc[:, idx, :])
                    eng = nc.sync if idx % 2 == 0 else nc.scalar
                    eng.dma_start(
                        y_t.ap()[ch * P:(ch + 1) * P, iq * 1024:(iq + 1) * 1024],
                        y_sb[:],
                    )

            G = [list(range(0, 4)), list(range(4, 8)), list(range(8, 12)),
                 list(range(12, 15)), [15]]

            for jt in range(NJT):
                for iq in range(4):
                    if jt < 4 and iq < 3:
                        continue  # pre-emitted during the qkv phase
                    s_tile(jt, iq)
                # per-jt denominator (sum the 4 chunk sums) + vp scaling
                nc.vector.reduce_sum(
                    dsum_all[:, jt:jt + 1], d_all[:, jt * 4:jt * 4 + 4],
                    axis=mybir.AxisListType.X,
                )
                nc.vector.reciprocal(
                    dsum_all[:, 16 + jt:17 + jt], dsum_all[:, jt:jt + 1]
                )
                nc.vector.tensor_scalar_mul(
                    v_all[:, jt, :], v_all[:, jt, :],
                    dsum_all[:, 16 + jt:17 + jt],
                )
                if 4 <= jt < 12:
                    # groups g0-g1: two chains per jt (8 chains over 4 jts)
                    g = jt // 4 - 1
                    off = (jt % 4) * 2
                    out_chain(G[g], g, off)
                    out_chain(G[g], g, off + 1)
                elif 12 <= jt < 14:
                    # group g2 compressed to 4 chains per jt (done by jt13)
                    for off in range(4 * (jt - 12), 4 * (jt - 12) + 4):
                        out_chain(G[2], 2, off)
                elif jt == 15:
                    # g3a (jts 12-14) interleaved with jt15's score tiles
                    for idx in range(8):
                        out_chain(G[3], 3, idx)
            # final: single-jt chains, then add + store immediately
            for idx in range(8):
                out_chain(G[4], 4, idx, final=True)

_nc_cache = None
LAST_EXEC_TIME_NS = None


def bp_ap(t, ap, off=0):
    return bass.AP(tensor=t, offset=off, ap=ap)


def _get_nc():
    global _nc_cache
    if _nc_cache is None:
        _nc_cache = _build_module()
    return _nc_cache


def _pack_x(xb):
    # xb [C, N] bf16 -> [128, sum(2*w)] with per-block [p, ci, w] layout
    cols = []
    for lo, w in XBLK:
        blkv = xb[:, lo:lo + w].reshape(2, P, w).transpose(1, 0, 2)
        cols.append(np.ascontiguousarray(blkv).reshape(P, 2 * w))
    return np.ascontiguousarray(np.concatenate(cols, axis=1))


def kernel(x, wq, bq, wk, bk, wv, bv, wp, bp):
    global LAST_EXEC_TIME_NS
    nc = _get_nc()

    import ml_dtypes
    bf = ml_dtypes.bfloat16
    x = np.asarray(x, dtype=np.float32).reshape(B, C, N).astype(bf)
    wq32 = np.asarray(wq, dtype=np.float32)
    wk32 = np.asarray(wk, dtype=np.float32)
    wv32 = np.asarray(wv, dtype=np.float32)
    wp32 = np.asarray(wp, dtype=np.float32)
    w2 = wp32 @ wv32                      # fold the output projection into v

    # w_pack[p, 2*w+ci, co] = wT[w][ci*128+p, co]
    wT = np.stack([wq32.T, wk32.T, w2.T]).astype(bf)    # [3, 256 ci, 256 co]
    w_pack = np.zeros((P, 8, C), dtype=bf)
    for w in range(3):
        w_pack[:, 2 * w:2 * w + 2, :] = wT[w].reshape(2, P, C).transpose(1, 0, 2)
    w_pack = np.ascontiguousarray(w_pack.reshape(P, 8 * C))

    # b_pack cols 0-3: [bq_lo, bq_hi, bk_lo, bk_hi] per partition; 4:260: bv2
    bq32 = np.asarray(bq, dtype=np.float32)
    bk32 = np.asarray(bk, dtype=np.float32)
    b_pack = np.zeros((P, 4 + C), dtype=np.float32)
    b_pack[:, 0] = bq32[:P]
    b_pack[:, 1] = bq32[P:]
    b_pack[:, 2] = bk32[:P]
    b_pack[:, 3] = bk32[P:]
    b_pack[:, 4:] = (wp32 @ np.asarray(bv, dtype=np.float32))[None, :]
    b_pack = np.ascontiguousarray(b_pack)

    bp1 = np.asarray(bp, dtype=np.float32).reshape(C)

    in_maps = []
    for core in range(8):
        b, h = divmod(core, 2)
        xb = x[b] if h == 0 else np.ascontiguousarray(np.roll(x[b], -NJ, axis=1))
        in_maps.append({"xp": _pack_x(xb), "wp": w_pack, "bp": b_pack})

    res = bass_utils.run_bass_kernel_spmd(nc, in_maps, core_ids=list(range(8)))
    if res.exec_time_ns is not None:
        LAST_EXEC_TIME_NS = res.exec_time_ns

    y = np.zeros((B, C, N), np.float32)
    for b in range(B):
        y[b] = res.results[2 * b]["y"] + np.roll(res.results[2 * b + 1]["y"], NJ, axis=1)
    y += bp1.reshape(1, C, 1)
    return y.reshape(B, C, 64, 64)
